# revision 42
# baseline (speedup 1.0000x reference)
"""GPT-2 attention block on 8 TRN2 NeuronCores.

Sharding (Megatron-style): core i owns heads (2i, 2i+1) for both batches.
 - QKV projection computed transposed: qkvT = Wshard^T @ X^T  -> [384, 4096]
   (rows: q0|q1|k0|k1|v0|v1 head-dim slices, cols: tokens b-major)
 - scores computed transposed per (batch, head): S^T[k, q] (causal tiles only),
   exp on ScalarE (scale 1/8 folded in), denominator via ones-matmul on PE,
   AV with V natural (from PE transposes of V^T) as the stationary operand,
   both heads packed into one PSUM via col-tiling.
 - av normalized+transposed to natural [token, dcol] layout, AllToAll ->
   sequence parallel: each core gets its 512-token slice of all 1024 dcols,
   computes the full output projection for those tokens.
Output per core: [512, 1024] fp32 token slice; host concatenates.
Matmul compute in bf16 (fp32 accumulation in PSUM); softmax math in fp32.
"""

import numpy as np
import ml_dtypes

import concourse.bass as bass
import concourse.mybir as mybir
import concourse.tile as tile
from concourse.bass_utils import run_bass_kernel_spmd

BF16 = mybir.dt.bfloat16
F32 = mybir.dt.float32
AF = mybir.ActivationFunctionType

B, S, D, H = 2, 2048, 1024, 16
NT = B * S          # 4096 tokens, b-major
NCORES = 8
DK = D // H         # 64
NEG = -1.0e30
SCALE = 0.125       # 1/sqrt(64)

_CACHE = {}
_NO_COLLECTIVE = False


def _build(debug_dumps=False):
    nc = bass.Bass("TRN2", target_bir_lowering=False, debug=False,
                   num_devices=NCORES)

    xT = nc.dram_tensor("xT", [D, NT], BF16, kind="ExternalInput").ap()
    wqkv = nc.dram_tensor("wqkv", [D, 384], BF16, kind="ExternalInput").ap()
    wp = nc.dram_tensor("wp", [D, D], BF16, kind="ExternalInput").ap()
    cbf16 = nc.dram_tensor("cbf16", [128, 129], BF16, kind="ExternalInput").ap()
    cf32 = nc.dram_tensor("cf32", [128, 131], F32, kind="ExternalInput").ap()
    out = nc.dram_tensor("out", [512, 1024], F32, kind="ExternalOutput").ap()
    dbg = None
    if debug_dumps:
        dbg = {
            "dbg_qkvT": nc.dram_tensor(
                "dbg_qkvT", [128, 3, NT], BF16, kind="ExternalOutput").ap(),
            "dbg_av": nc.dram_tensor(
                "dbg_av", [128, 32, 128], BF16, kind="ExternalOutput").ap(),
            "dbg_den": nc.dram_tensor(
                "dbg_den", [8, 128, 8], F32, kind="ExternalOutput").ap(),
            "dbg_aT": nc.dram_tensor(
                "dbg_aT", [128, 8, 512], BF16, kind="ExternalOutput").ap(),
        }

    with tile.TileContext(nc) as tc:
        _body(tc, out, xT, wqkv, wp, cbf16, cf32, dbg)
    _dedup_ldweights(nc)
    _split_multi_waits(nc)
    return nc


def _dedup_ldweights(nc):
    """Drop a back-to-back identical, wait-free Ldweights (weights already
    resident; only Matmults in between; transposes clobber -> reset)."""
    for f in nc.m.functions:
        for bb in f.blocks:
            insts = bb.instructions
            new = []
            changed = False
            last_w = None
            for inst in insts:
                nm = inst.__class__.__name__
                if getattr(inst, "engine", None) == mybir.EngineType.PE:
                    if nm == "InstLdweights":
                        si = inst.sync_info
                        key = repr(inst.ins)
                        no_waits = si is None or not si.on_wait
                        no_upd = si is None or not si.on_update
                        if key == last_w and no_waits and no_upd:
                            changed = True
                            continue  # drop duplicate load
                        last_w = key
                    elif nm == "InstMatmult":
                        if getattr(inst, "is_transpose", False):
                            last_w = None
                    else:
                        last_w = None
                new.append(inst)
            if changed:
                bb.instructions = new


def _split_multi_waits(nc):
    """Walrus caps HW sync waits at 1 per instruction: hoist extras onto
    dedicated NoOps inserted just before the offender (same engine queue)."""
    import bass_rust
    nid = [0]
    for f in nc.m.functions:
        for bb in f.blocks:
            insts = bb.instructions
            new = []
            changed = False
            for inst in insts:
                si = getattr(inst, "sync_info", None)
                if si is not None and len(si.on_wait) > 1:
                    changed = True
                    waits = list(si.on_wait)
                    for w in waits[:-1]:
                        nid[0] += 1
                        nop = mybir.InstNoOp(
                            name=f"I-waitnop-{nid[0]}", ins=[], outs=[])
                        nop.engine = inst.engine
                        nop.sync_info = bass_rust.SyncInfo(
                            on_wait=[w], on_update=[])
                        new.append(nop)
                    inst.sync_info = bass_rust.SyncInfo(
                        on_wait=[waits[-1]], on_update=list(si.on_update))
                new.append(inst)
            if changed:
                bb.instructions = new


def _body(tc, out, xT, wqkv, wp, cbf16, cf32, dbg=None):
    nc = tc.nc

    with (
        tc.tile_pool(name="persist", bufs=1) as persist,
        tc.tile_pool(name="expp", bufs=24) as expp_pool,
        tc.tile_pool(name="avts", bufs=2) as avts_pool,
        tc.tile_pool(name="dens", bufs=2) as dens_pool,
        tc.tile_pool(name="smalls", bufs=4) as smalls_pool,
        tc.tile_pool(name="ablk", bufs=2) as ablk_pool,
        tc.tile_pool(name="outs", bufs=3) as outs_pool,
        tc.tile_pool(name="pss", bufs=2, space="PSUM") as pss_pool,
        tc.tile_pool(name="qkvp", bufs=2, space="PSUM") as qkvp_pool,
        tc.tile_pool(name="psa", bufs=2, space="PSUM") as psa_pool,
        tc.tile_pool(name="dram", bufs=1, space="DRAM") as dram_pool,
    ):
        # ---- persistent SBUF ----
        xT_sb = persist.tile([128, 8, NT], BF16)        # X^T, D-tile major
        wqkv_sb = persist.tile([128, 8, 384], BF16)
        wp_sb = persist.tile([128, 8, 1024], BF16)
        qkvT_sb = persist.tile([128, 3, NT], BF16)      # q|k|v ^T rows
        v_aug = persist.tile([128, 32, 130], BF16)      # [v_h0|1|v_h1|1] per token-tile
        av_sb = persist.tile([128, 32, 128], BF16)      # av natural, per token-tile
        aT_sb = persist.tile([128, 8, 512], BF16)       # a^T after all-to-all
        cbf16_sb = persist.tile([128, 129], BF16)
        cf32_sb = persist.tile([128, 131], F32)
        ident_sb = cbf16_sb[:, 0:128]
        ones_sb = cbf16_sb[:, 128:129]
        maskT_sb = cf32_sb[:, 0:128]
        bqkv_sb = cf32_sb[:, 128:131]

        av_bounce = [dram_pool.tile([S, 128], BF16, name=f"avb{b}")
                     for b in range(2)]
        recv_bounce = [dram_pool.tile([S, 128], BF16, name=f"rcv{b}")
                       for b in range(2)]

        # ones columns of v_aug (rest overwritten by V transposes)
        nc.vector.memset(v_aug[:, :, 64:65], 1.0)
        nc.vector.memset(v_aug[:, :, 129:130], 1.0)

        # ---- ACT warmup: attach table-load pseudos to wait-free instructions
        warm = smalls_pool.tile([1, 2], F32, tag="warm")
        nc.vector.memset(warm[:, 0:1], 0.0)
        nc.scalar.activation(warm[:, 1:2], warm[:, 0:1], AF.Identity)
        nc.scalar.activation(warm[:, 1:2], warm[:, 0:1], AF.Exp)
        nc.scalar.activation(warm[:, 1:2], warm[:, 0:1], AF.Copy)

        # ---- input DMAs: xT chunked by TOKENS so each chunk's qkv (full
        # D contraction) completes as soon as that chunk lands.
        nc.sync.dma_start(wqkv_sb[:, :, :],
                          wqkv.rearrange("(kt p) n -> p kt n", p=128))
        nc.sync.dma_start(cbf16_sb[:, :], cbf16[:, :])
        nc.sync.dma_start(cf32_sb[:, :], cf32[:, :])
        for n in range(8):
            nc.sync.dma_start(
                xT_sb[:, :, n * 512:(n + 1) * 512],
                xT[:, n * 512:(n + 1) * 512]
                .rearrange("(kt p) w -> p kt w", p=128))
        nc.sync.dma_start(wp_sb[:, :, :],
                          wp.rearrange("(kt p) n -> p kt n", p=128))

        # ---- phase 1: qkvT = Wshard^T @ X^T, bias add, bf16 ----
        # token-chunk outer; V transposes interleaved per chunk.
        for n in range(8):
            for m in range(3):
                ps = qkvp_pool.tile([128, 512], F32, tag="qkvp")
                for kt in range(8):
                    nc.tensor.matmul(
                        ps[:, :],
                        wqkv_sb[:, kt, m * 128:(m + 1) * 128],
                        xT_sb[:, kt, n * 512:(n + 1) * 512],
                        start=(kt == 0), stop=(kt == 7),
                    )
                nc.vector.tensor_scalar_add(
                    qkvT_sb[:, m, n * 512:(n + 1) * 512],
                    ps[:, :], bqkv_sb[:, m:m + 1])
            for t in range(n * 4, n * 4 + 4):
                ps_t = psa_pool.tile([128, 128], BF16, tag="psa")
                nc.tensor.transpose(
                    ps_t[:, :], qkvT_sb[:, 2, t * 128:(t + 1) * 128],
                    ident_sb[:, :])
                nc.vector.tensor_copy(v_aug[:, t, 0:64], ps_t[:, 0:64])
                nc.vector.tensor_copy(v_aug[:, t, 65:129], ps_t[:, 64:128])

        def attention(b):
            tok0 = b * S
            for c in range(4):
                nk = 4 * c + 4        # k-tiles 0..nk-1
                q0 = tok0 + c * 512   # global col of chunk start
                expp = {}
                for ki in range(nk):
                    off = max(0, (ki - 4 * c)) * 128
                    w = 512 - off
                    ps_s = pss_pool.tile([128, 2, 512], F32, tag="pss")
                    for h in range(2):
                        hp = h * 64
                        nc.tensor.matmul(
                            ps_s[:, h, :w],
                            qkvT_sb[hp:hp + 64, 1,
                                    tok0 + ki * 128: tok0 + (ki + 1) * 128],
                            qkvT_sb[hp:hp + 64, 0, q0 + off: q0 + 512],
                            start=True, stop=True,
                        )
                    if ki >= 4 * c:  # diagonal tile: causal mask on first 128
                        for h in range(2):
                            nc.vector.tensor_add(
                                ps_s[:, h, 0:128], ps_s[:, h, 0:128],
                                maskT_sb[:, :])
                    et = expp_pool.tile([128, 2, 512], BF16, tag="expp")
                    nc.scalar.activation(
                        et[:, :, :w], ps_s[:, :, :w], AF.Exp, scale=SCALE)
                    expp[ki] = (et, off, w)

                # denominator + AV (col-packed heads for AV)
                den_sb = dens_pool.tile([1, 1024], F32, tag="dens")
                avT_sbuf = avts_pool.tile([128, 512], BF16, tag="avts")
                for h in range(2):
                    avh_ps = psa_pool.tile([65, 512], F32, tag="psa")
                    for ki in range(nk):
                        et, off, w = expp[ki]
                        nc.tensor.matmul(
                            avh_ps[:, off:512],
                            v_aug[:, b * 16 + ki, h * 65:(h + 1) * 65],
                            et[:, h, :w],
                            start=(ki == 0), stop=(ki == nk - 1),
                        )
                    nc.vector.tensor_copy(
                        avT_sbuf[h * 64:(h + 1) * 64, :], avh_ps[0:64, :])
                    nc.vector.tensor_copy(den_sb[0:1, h * 512:(h + 1) * 512],
                                          avh_ps[64:65, :])

                # reshape denominators [1, h*512+qt*128+p] -> [p, h*4+qt]
                # (via DRAM: SBUF APs cannot move free offsets onto partitions)
                den_dram = dram_pool.tile([1, 1024], F32, tag="dend", bufs=2)
                nc.sync.dma_start(den_dram[:, :], den_sb[0:1, :])
                den_col = smalls_pool.tile([128, 8], F32, tag="denc")
                nc.sync.dma_start(
                    den_col[:, :],
                    den_dram.rearrange("a (hq p) -> (a p) hq", p=128))
                recip_col = smalls_pool.tile([128, 8], F32, tag="recipc")
                nc.vector.reciprocal(recip_col[:, :], den_col[:, :])
                if dbg is not None:
                    nc.sync.dma_start(dbg["dbg_den"][b * 4 + c, :, :],
                                      den_col[:, :])

                # transpose av^T -> natural, normalize per head
                for qt in range(4):
                    ps_t = psa_pool.tile([128, 128], BF16, tag="psa")
                    nc.tensor.transpose(
                        ps_t[:, :], avT_sbuf[:, qt * 128:(qt + 1) * 128],
                        ident_sb[:, :])
                    tindex = b * 16 + c * 4 + qt
                    for h in range(2):
                        hp = h * 64
                        nc.vector.tensor_scalar_mul(
                            av_sb[:, tindex, hp:hp + 64], ps_t[:, hp:hp + 64],
                            recip_col[:, h * 4 + qt: h * 4 + qt + 1])

        def a2a(b):
            # all-to-all over this batch: 256-token blocks to each core
            nc.sync.dma_start(
                av_bounce[b].rearrange("(t p) d -> p t d", p=128),
                av_sb[:, b * 16:(b + 1) * 16, :])
            if _NO_COLLECTIVE:
                nc.sync.dma_start(recv_bounce[b][:, :], av_bounce[b][:, :])
            else:
                nc.gpsimd.collective_compute(
                    "AllToAll", mybir.AluOpType.bypass,
                    replica_groups=[list(range(NCORES))],
                    ins=[av_bounce[b][:, :].opt()],
                    outs=[recv_bounce[b][:, :].opt()],
                )

        def recv_stage(b):
            # rebuild a^T [dcol, 256 tok of batch b]
            a_stg = ablk_pool.tile([128, 16, 128], BF16, tag="ablk")
            nc.sync.dma_start(
                a_stg[:, :, :],
                recv_bounce[b].rearrange("(t p) d -> p t d", p=128))
            for s in range(8):
                for j in range(2):
                    ps_t = qkvp_pool.tile([128, 128], BF16, tag="qkvp")
                    nc.tensor.transpose(ps_t[:, :], a_stg[:, s * 2 + j, :],
                                        ident_sb[:, :])
                    nc.vector.tensor_copy(
                        aT_sb[:, s, b * 256 + j * 128: b * 256 + (j + 1) * 128],
                        ps_t[:, :])

        def proj(b):
            for mt in range(2):
                r0 = b * 256 + mt * 128
                for n2 in range(2):
                    ps = qkvp_pool.tile([128, 512], F32, tag="qkvp")
                    for s in range(8):
                        nc.tensor.matmul(
                            ps[:, :],
                            aT_sb[:, s, r0:r0 + 128],
                            wp_sb[:, s, n2 * 512:(n2 + 1) * 512],
                            start=(s == 0), stop=(s == 7),
                        )
                    o_sb = outs_pool.tile([128, 512], F32, tag="outs")
                    nc.vector.tensor_copy(o_sb[:, :], ps[:, :])
                    nc.sync.dma_start(
                        out[r0:r0 + 128, n2 * 512:(n2 + 1) * 512],
                        o_sb[:, :])

        attention(0)
        a2a(0)
        recv_stage(0)
        attention(1)
        proj(0)
        a2a(1)
        recv_stage(1)
        proj(1)

        if dbg is not None:
            nc.sync.dma_start(
                dbg["dbg_qkvT"].rearrange("p m n -> p (m n)"),
                qkvT_sb[:, :, :].rearrange("p m n -> p (m n)"))
            nc.sync.dma_start(
                dbg["dbg_av"].rearrange("p t d -> p (t d)"),
                av_sb[:, :, :].rearrange("p t d -> p (t d)"))
            nc.sync.dma_start(
                dbg["dbg_aT"].rearrange("p s n -> p (s n)"),
                aT_sb[:, :, :].rearrange("p s n -> p (s n)"))


def _prep_inputs(hidden_states, c_attn_w, c_attn_b, c_proj_w):
    bf16 = ml_dtypes.bfloat16
    x = np.asarray(hidden_states, dtype=np.float32).reshape(NT, D)
    xT = np.ascontiguousarray(x.T).astype(bf16)
    wp = np.ascontiguousarray(np.asarray(c_proj_w, dtype=np.float32)).astype(bf16)
    identity = np.eye(128, dtype=np.float32).astype(bf16)
    ones = np.ones((128, 1), dtype=np.float32).astype(bf16)
    cbf16 = np.ascontiguousarray(np.concatenate([identity, ones], axis=1))
    # maskT[p, f]: S^T diagonal tile entry (k=p, q=f) masked iff q < k
    p = np.arange(128)
    maskT = np.where(p[None, :] >= p[:, None], 0.0, NEG).astype(np.float32)

    w = np.asarray(c_attn_w, dtype=np.float32)
    bb = np.asarray(c_attn_b, dtype=np.float32)
    in_maps = []
    for i in range(NCORES):
        cols = np.r_[i * 128:(i + 1) * 128]
        wshard = np.concatenate(
            [w[:, cols], w[:, D + cols], w[:, 2 * D + cols]], axis=1)
        bshard = np.stack(
            [bb[cols], bb[D + cols], bb[2 * D + cols]], axis=1)  # [128, 3]
        cf32 = np.ascontiguousarray(
            np.concatenate([maskT, bshard], axis=1)).astype(np.float32)
        in_maps.append({
            "xT": xT,
            "wqkv": np.ascontiguousarray(wshard).astype(bf16),
            "wp": wp,
            "cbf16": cbf16,
            "cf32": cf32,
        })
    return in_maps


def kernel(hidden_states, c_attn_w, c_attn_b, c_proj_w, c_proj_b, _trace=False):
    if "nc" not in _CACHE:
        _CACHE["nc"] = _build()
    nc = _CACHE["nc"]
    in_maps = _prep_inputs(hidden_states, c_attn_w, c_attn_b, c_proj_w)
    try:
        res = run_bass_kernel_spmd(nc, in_maps, core_ids=list(range(NCORES)),
                                   trace=_trace)
    except (ImportError, ModuleNotFoundError):
        # NTFF profiling hook unavailable in this container
        res = run_bass_kernel_spmd(nc, in_maps, core_ids=list(range(NCORES)),
                                   trace=False)
    _CACHE["last_result"] = res
    # core j's output rows: [0:256] = batch0 tokens 256j.., [256:512] = batch1
    full = np.empty((NT, D), dtype=np.float32)
    for j in range(NCORES):
        o = res.results[j]["out"]
        full[256 * j:256 * (j + 1)] = o[0:256]
        full[S + 256 * j:S + 256 * (j + 1)] = o[256:512]
    full = full + np.asarray(c_proj_b, dtype=np.float32)[None, :]
    return full.reshape(B, S, D).astype(np.float32)
